# revision 23
# baseline (speedup 1.0000x reference)
"""Trainium2 Bass kernel for nn_MatSurfGcn (GCN message passing, memory-bound).

Strategy (column-parallel over W_g1's output dim, 8 cores):
  reference =  enc -> gcn_conv(W_g1) -> gcn_conv(W_g2) -> head
  Both convs are linear and A @ (X @ W) == (A @ X) @ W, so the graph
  aggregation commutes out of the device entirely.  Per core c:
    x0T = relu(enc(inputs)).T          [4096, 14]  (host, replicated)
    z_c = x0T.T @ (W_g1[:,c] * w2_c)   [14, 1024]  (device, the real work)
    t_c = row_sum(z_c)                 [14, ..]
    host: y = head(A(A Su + b1.W_g2) + b_g2)       (two 14x14 matvecs)

  W_g2 is folded into W_g1's columns on the host (same device FLOPs, kills
  the tail multiply).  The folded weight streams with column-split mixed
  precision: after the fold, column j's contribution to t scales with
  |w2[j]|, so the 512 smallest-|w2| columns per core (~13% of the sum-w2^2
  energy) stream as scaled fp8-e4m3 and the 512 largest as bf16 --
  1.5 B/elem, 6.3 MB/core, ~16.5 us of DMA at the ~380 GB/s per-core HBM
  limit, against ~14 us of PE bf16 column stream.  Measured end-to-end
  error ~5e-3 vs the 2e-2 gate (quantization noise does not average down
  through the random-sign contraction, but starts 4x under the gate).

  The 14-node activations x0T are computed on the host and replicated to
  all cores (per the sharding hint): on the PE the encoder decomposes
  into 32 latency-bound 14-column matmuls (~13 us) that would gate the
  first z matmul and starve the W stream, while as data it is a 114 KiB
  bf16 ride-along on the first W tile's DMA.
"""

import os

import numpy as np

D1, D2 = 4096, 8192
N = 14
NCORES = 8
SH = D2 // NCORES        # 1024 W_g1 columns per core
KC = D1 // 128           # 32 contraction chunks of 128
NS = 512                 # fp8 (small-|w2|) columns per core; rest bf16
# 4-chunk super-tiles keep the DMA-issue count below the stream time;
# small tiles at the head start the PE sooner, small tiles at the end
# shorten the post-stream matmul chain
TILE_CHUNKS = [1, 1, 2] + [4] * 6 + [2, 1, 1]
KSPLIT = 28              # z accumulation groups: k<28 and k>=28 (reduce overlap)
WBUFS = int(os.environ.get("KERNEL_WBUFS", "8"))
XBC = KC * N             # xb columns, prepended to the bf16 image

_CACHE = {}


def _build_nc():
    import concourse.bacc as bacc
    import concourse.bass as bass
    import concourse.mybir as mybir
    import concourse.tile as tile

    f32 = mybir.dt.float32
    bf16 = mybir.dt.bfloat16
    fp8 = mybir.dt.float8e4
    psum = bass.MemorySpace.PSUM
    alu = mybir.AluOpType

    nc = bacc.Bacc(
        "TRN2", target_bir_lowering=False, debug=False, enable_asserts=False
    )

    # host-computed x0T = relu(enc(inputs)).T (swizzled so chunk k sits at
    # cols 14k..14k+14) prepended to the host-swizzled bf16 half of the
    # folded W_g1 shard; the scaled fp8 half is a second image:
    #   wv16[p, k*N + n]       = x0T[k*128 + p, n]
    #   wv16[p, XBC + k*NB + j] = (W_g1 * w2)[k*128+p, big cols]  as bf16
    #   wv8[p, k*NS + j]        = (W_g1 * w2 * s8)[k*128+p, small cols] as fp8
    NB = SH - NS
    wv16_d = nc.dram_tensor(
        "wv16", [128, XBC + KC * NB], bf16, kind="ExternalInput"
    )
    wv8_d = nc.dram_tensor("wv8", [128, KC * NS], fp8, kind="ExternalInput")
    t_d = nc.dram_tensor("t", [N, 4], f32, kind="ExternalOutput")

    with tile.TileContext(nc) as tc:
        with (
            tc.tile_pool(name="const", bufs=1) as cpool,
            tc.tile_pool(name="wvp", bufs=WBUFS) as wpool,
            tc.tile_pool(name="zps", bufs=1, space=psum) as zps,
        ):
            t2 = cpool.tile([N, 4], f32)
            z16a = zps.tile([N, NB], f32)
            z8a = zps.tile([N, NS], f32)
            z16b = zps.tile([N, NB], f32)
            z8b = zps.tile([N, NS], f32)

            # Tile 0 carries [xb | bf16 chunks 0..3] in ONE transfer; it
            # lives in the const pool so xb persists for all 64 stationary
            # loads.  Each super-tile is a bf16 + an fp8 transfer on the
            # same sync HWDGE ring: one ring keeps arrival order = need
            # order.
            t0w = TILE_CHUNKS[0] * NB
            tile0 = cpool.tile([128, XBC + t0w], bf16)
            nc.sync.dma_start(out=tile0[:], in_=wv16_d[:, 0 : XBC + t0w])
            xb = tile0[:, 0:XBC]
            w8_0 = wpool.tile([128, TILE_CHUNKS[0] * NS], fp8, tag="w8h")
            nc.sync.dma_start(
                out=w8_0[:], in_=wv8_d[:, 0 : TILE_CHUNKS[0] * NS]
            )
            tiles = [(tile0[:, XBC : XBC + t0w], w8_0[:], 0, TILE_CHUNKS[0])]
            k0 = TILE_CHUNKS[0]
            for nch in TILE_CHUNKS[1:]:
                w16 = wpool.tile([128, nch * NB], bf16, tag=f"w16{nch}")
                nc.sync.dma_start(
                    out=w16[:],
                    in_=wv16_d[:, XBC + k0 * NB : XBC + (k0 + nch) * NB],
                )
                w8 = wpool.tile([128, nch * NS], fp8, tag=f"w8{nch}")
                nc.sync.dma_start(
                    out=w8[:], in_=wv8_d[:, k0 * NS : (k0 + nch) * NS]
                )
                tiles.append((w16[:], w8[:], k0, nch))
                k0 += nch

            # z += x0T_k.T @ Wv_k, accumulated in PSUM per (dtype, k-group);
            # the k<28 groups' reduces overlap the last chunks' matmuls
            for w16, w8, k0, nch in tiles:
                for a in range(nch):
                    k = k0 + a
                    z16 = z16a if k < KSPLIT else z16b
                    z8 = z8a if k < KSPLIT else z8b
                    st = k in (0, KSPLIT)
                    sp = k in (KSPLIT - 1, KC - 1)
                    nc.tensor.matmul(
                        z16[:],
                        xb[:, k * N : (k + 1) * N],
                        w16[:, a * NB : (a + 1) * NB],
                        start=st,
                        stop=sp,
                    )
                    nc.tensor.matmul(
                        z8[:],
                        xb[:, k * N : (k + 1) * N],
                        w8[:, a * NS : (a + 1) * NS],
                        start=st,
                        stop=sp,
                    )

            # t = per-group row_sums; host adds the four columns (fp8 ones
            # descaled).  Group-A reduces run on DVE while group B finishes
            # on the PE, so only group B's reduces sit in the tail.
            nc.vector.tensor_reduce(
                t2[:, 0:1], z16a[:], axis=mybir.AxisListType.X, op=alu.add
            )
            nc.vector.tensor_reduce(
                t2[:, 1:2], z8a[:], axis=mybir.AxisListType.X, op=alu.add
            )
            nc.vector.tensor_reduce(
                t2[:, 2:3], z16b[:], axis=mybir.AxisListType.X, op=alu.add
            )
            nc.vector.tensor_reduce(
                t2[:, 3:4], z8b[:], axis=mybir.AxisListType.X, op=alu.add
            )
            nc.sync.dma_start(out=t_d[:], in_=t2[:])

    nc.compile()
    return nc


def get_nc():
    if "nc" not in _CACHE:
        _CACHE["nc"] = _build_nc()
    return _CACHE["nc"]


def build_graph_matrix(edge_index):
    """Dense normalized adjacency of the PyG-style GCNConv (self-loops +
    symmetric deg^{-1/2}); multi-edges accumulate like segment_sum does."""
    ei = np.concatenate(
        [edge_index.astype(np.int64), np.stack([np.arange(N), np.arange(N)])],
        axis=1,
    )
    src, dst = ei[0], ei[1]
    deg = np.zeros(N, np.float32)
    np.add.at(deg, dst, np.ones(len(dst), np.float32))
    dis = np.where(deg > 0, 1.0 / np.sqrt(np.maximum(deg, 1e-12)), 0.0).astype(
        np.float32
    )
    A = np.zeros((N, N), np.float32)
    np.add.at(A, (dst, src), dis[src] * dis[dst])
    return A


def build_host_inputs(inputs):
    """Per-core input maps + graph matrix + global fp8 scale."""
    f32 = np.float32
    import ml_dtypes

    bf16 = ml_dtypes.bfloat16
    e4m3 = ml_dtypes.float8_e4m3fn
    mats = np.asarray(inputs["mats"], f32)
    cyls = np.asarray(inputs["cyls"], f32)
    planes = np.asarray(inputs["planes"], f32)
    power = np.asarray(inputs["power"], f32)
    edge_index = np.asarray(inputs["edge_index"])

    A = build_graph_matrix(edge_index)

    # Block-diagonal node features with bias rows of ones: x0 = relu(S.T @ Wenc)
    ENC_K = 18
    S = np.zeros((ENC_K, N), f32)
    S[0:6, 0:6] = mats.T
    S[6, 0:6] = 1.0
    S[7:10, 6:10] = cyls.T
    S[10, 6:10] = 1.0
    S[11:15, 10:13] = planes.T
    S[15, 10:13] = 1.0
    S[16, 13] = power[0] / 10000.0
    S[17, 13] = 1.0

    Wenc = np.ascontiguousarray(
        np.concatenate(
            [
                np.asarray(inputs["W_mat"], f32),
                np.asarray(inputs["b_mat"], f32)[None, :],
                np.asarray(inputs["W_cyl"], f32),
                np.asarray(inputs["b_cyl"], f32)[None, :],
                np.asarray(inputs["W_pl"], f32),
                np.asarray(inputs["b_pl"], f32)[None, :],
                np.asarray(inputs["W_pw"], f32),
                np.asarray(inputs["b_pw"], f32)[None, :],
            ],
            axis=0,
        )
    )
    assert Wenc.shape == (ENC_K, D1)

    W_g1 = np.asarray(inputs["W_g1"], f32)
    W_g2 = np.asarray(inputs["W_g2"], f32)

    # replicated 14-node activations, transposed + chunk-swizzled for the PE
    x0 = np.maximum(S.T @ Wenc, 0.0)  # [N, D1]
    xb = np.ascontiguousarray(
        x0.T.reshape(KC, 128, N).transpose(1, 0, 2).reshape(128, KC * N)
    ).astype(bf16)

    NB = SH - NS
    per_core = []
    gmax = 0.0
    for c in range(NCORES):
        sl = slice(c * SH, (c + 1) * SH)
        w2 = W_g2[sl, 0]
        order = np.argsort(np.abs(w2))
        idx8, idx16 = order[:NS], order[NS:]
        Wv = W_g1[:, sl] * w2[None, :]  # [D1, SH]
        W8 = Wv[:, idx8]
        W16 = Wv[:, idx16]
        gmax = max(gmax, float(np.abs(W8).max()))
        per_core.append((W8, W16))
    s8 = float(2.0 ** np.floor(np.log2(224.0 / max(gmax, 1e-30))))

    def swz(w, ncols):
        return w.reshape(KC, 128, ncols).transpose(1, 0, 2).reshape(128, -1)

    in_maps = []
    for c in range(NCORES):
        W8, W16 = per_core[c]
        wv16 = np.ascontiguousarray(
            np.concatenate([xb, swz(W16.astype(bf16), NB)], axis=1)
        )
        wv8 = np.ascontiguousarray(swz((W8 * s8).astype(e4m3), NS))
        in_maps.append({"wv16": wv16, "wv8": wv8})
    return in_maps, A, s8


def combine_t(t_raw, s8):
    """Device t [14,4] = [bf16 A, fp8 A, bf16 B, fp8 B] -> one [14,1] part."""
    t = np.asarray(t_raw, np.float64)
    u = t[:, 0] + t[:, 2] + (t[:, 1] + t[:, 3]) / s8
    return u[:, None].astype(np.float32)


def epilogue(t_parts, A, inputs):
    f32 = np.float32
    b_g1 = np.asarray(inputs["b_g1"], f32)
    W_g2 = np.asarray(inputs["W_g2"], f32)
    b_g2 = np.asarray(inputs["b_g2"], f32)
    W_head = np.asarray(inputs["W_head"], f32)
    b_head = np.asarray(inputs["b_head"], f32)
    u = np.add.reduce([p.astype(f32) for p in t_parts])  # [14,1] un-aggregated
    t_full = A @ u + np.float32(b_g1 @ W_g2[:, 0])  # conv2 input = x1 @ W_g2
    x2 = A @ t_full + b_g2[0]
    y = float(x2[:, 0] @ W_head[:, 0]) + float(b_head[0])
    return np.array([y], dtype=f32)


def run_on_hw(in_maps, trace=False, tmpdir=None):
    from concourse.bass_utils import run_bass_kernel_spmd

    nc = get_nc()
    return run_bass_kernel_spmd(
        nc,
        in_maps,
        core_ids=list(range(NCORES)),
        trace=trace,
        tmpdir=tmpdir,
    )


def kernel(**inputs):
    in_maps, A, s8 = build_host_inputs(inputs)
    res = run_on_hw(in_maps, trace=bool(int(os.environ.get("KERNEL_TRACE", "0"))))
    _CACHE["last_result"] = res
    t_parts = [combine_t(r["t"], s8) for r in res.results]
    return epilogue(t_parts, A, inputs)


# revision 24
# speedup vs baseline: 1.1061x; 1.1061x over previous
"""Trainium2 Bass kernel for nn_MatSurfGcn (GCN message passing, memory-bound).

Strategy (column-parallel over W_g1's output dim, 8 cores):
  reference =  enc -> gcn_conv(W_g1) -> gcn_conv(W_g2) -> head
  Both convs are linear and A @ (X @ W) == (A @ X) @ W, so the graph
  aggregation commutes out of the device entirely.  Per core c:
    x0T = relu(enc(inputs)).T          [4096, 14]  (host, replicated)
    z_c = x0T.T @ (W_g1[:,c] * w2_c)   [14, 1024]  (device, the real work)
    t_c = row_sum(z_c)                 [14, ..]
    host: y = head(A(A Su + b1.W_g2) + b_g2)       (two 14x14 matvecs)

  W_g2 is folded into W_g1's columns on the host (same device FLOPs, kills
  the tail multiply).  The folded weight streams with column-split mixed
  precision: after the fold, column j's contribution to t scales with
  |w2[j]|, so the 512 smallest-|w2| columns per core (~13% of the sum-w2^2
  energy) stream as scaled fp8-e4m3 and the 512 largest as bf16 --
  1.5 B/elem, 6.3 MB/core, ~16.5 us of DMA at the ~380 GB/s per-core HBM
  limit, against ~14 us of PE bf16 column stream.  Measured end-to-end
  error ~5e-3 vs the 2e-2 gate (quantization noise does not average down
  through the random-sign contraction, but starts 4x under the gate).

  The 14-node activations x0T are computed on the host and replicated to
  all cores (per the sharding hint): on the PE the encoder decomposes
  into 32 latency-bound 14-column matmuls (~13 us) that would gate the
  first z matmul and starve the W stream, while as data it is a 114 KiB
  bf16 ride-along on the first W tile's DMA.
"""

import os

import numpy as np

D1, D2 = 4096, 8192
N = 14
NCORES = 8
SH = D2 // NCORES        # 1024 W_g1 columns per core
KC = D1 // 128           # 32 contraction chunks of 128
NS = 512                 # fp8 (small-|w2|) columns per core; rest bf16
# 4-chunk super-tiles keep the DMA-issue count below the stream time
# (each super-tile costs two ~0.65us HWDGE issues on one sequencer, so
# many small tiles make the head issue-bound); small tiles at the end
# shorten the post-stream matmul chain
TILE_CHUNKS = [4] * 7 + [2, 1, 1]
KSPLIT = 28              # z accumulation groups: k<28 and k>=28 (reduce overlap)
WBUFS = int(os.environ.get("KERNEL_WBUFS", "8"))
XBC = KC * N             # xb columns, prepended to the bf16 image

_CACHE = {}


def _build_nc():
    import concourse.bacc as bacc
    import concourse.bass as bass
    import concourse.mybir as mybir
    import concourse.tile as tile

    f32 = mybir.dt.float32
    bf16 = mybir.dt.bfloat16
    fp8 = mybir.dt.float8e4
    psum = bass.MemorySpace.PSUM
    alu = mybir.AluOpType

    nc = bacc.Bacc(
        "TRN2", target_bir_lowering=False, debug=False, enable_asserts=False
    )

    # host-computed x0T = relu(enc(inputs)).T (swizzled so chunk k sits at
    # cols 14k..14k+14) prepended to the host-swizzled bf16 half of the
    # folded W_g1 shard; the scaled fp8 half is a second image:
    #   wv16[p, k*N + n]       = x0T[k*128 + p, n]
    #   wv16[p, XBC + k*NB + j] = (W_g1 * w2)[k*128+p, big cols]  as bf16
    #   wv8[p, k*NS + j]        = (W_g1 * w2 * s8)[k*128+p, small cols] as fp8
    NB = SH - NS
    wv16_d = nc.dram_tensor(
        "wv16", [128, XBC + KC * NB], bf16, kind="ExternalInput"
    )
    wv8_d = nc.dram_tensor("wv8", [128, KC * NS], fp8, kind="ExternalInput")
    t_d = nc.dram_tensor("t", [N, 4], f32, kind="ExternalOutput")

    with tile.TileContext(nc) as tc:
        with (
            tc.tile_pool(name="const", bufs=1) as cpool,
            tc.tile_pool(name="wvp", bufs=WBUFS) as wpool,
            tc.tile_pool(name="zps", bufs=1, space=psum) as zps,
        ):
            t2 = cpool.tile([N, 4], f32)
            z16a = zps.tile([N, NB], f32)
            z8a = zps.tile([N, NS], f32)
            z16b = zps.tile([N, NB], f32)
            z8b = zps.tile([N, NS], f32)

            # Tile 0 carries [xb | bf16 chunks 0..3] in ONE transfer; it
            # lives in the const pool so xb persists for all 64 stationary
            # loads.  Each super-tile is a bf16 + an fp8 transfer on the
            # same sync HWDGE ring: one ring keeps arrival order = need
            # order.
            t0w = TILE_CHUNKS[0] * NB
            tile0 = cpool.tile([128, XBC + t0w], bf16)
            nc.sync.dma_start(out=tile0[:], in_=wv16_d[:, 0 : XBC + t0w])
            xb = tile0[:, 0:XBC]
            w8_0 = wpool.tile([128, TILE_CHUNKS[0] * NS], fp8, tag="w8h")
            nc.sync.dma_start(
                out=w8_0[:], in_=wv8_d[:, 0 : TILE_CHUNKS[0] * NS]
            )
            tiles = [(tile0[:, XBC : XBC + t0w], w8_0[:], 0, TILE_CHUNKS[0])]
            k0 = TILE_CHUNKS[0]
            for nch in TILE_CHUNKS[1:]:
                w16 = wpool.tile([128, nch * NB], bf16, tag=f"w16{nch}")
                nc.sync.dma_start(
                    out=w16[:],
                    in_=wv16_d[:, XBC + k0 * NB : XBC + (k0 + nch) * NB],
                )
                w8 = wpool.tile([128, nch * NS], fp8, tag=f"w8{nch}")
                nc.sync.dma_start(
                    out=w8[:], in_=wv8_d[:, k0 * NS : (k0 + nch) * NS]
                )
                tiles.append((w16[:], w8[:], k0, nch))
                k0 += nch

            # z += x0T_k.T @ Wv_k, accumulated in PSUM per (dtype, k-group);
            # the k<28 groups' reduces overlap the last chunks' matmuls
            for w16, w8, k0, nch in tiles:
                for a in range(nch):
                    k = k0 + a
                    z16 = z16a if k < KSPLIT else z16b
                    z8 = z8a if k < KSPLIT else z8b
                    st = k in (0, KSPLIT)
                    sp = k in (KSPLIT - 1, KC - 1)
                    nc.tensor.matmul(
                        z16[:],
                        xb[:, k * N : (k + 1) * N],
                        w16[:, a * NB : (a + 1) * NB],
                        start=st,
                        stop=sp,
                    )
                    nc.tensor.matmul(
                        z8[:],
                        xb[:, k * N : (k + 1) * N],
                        w8[:, a * NS : (a + 1) * NS],
                        start=st,
                        stop=sp,
                    )

            # t = per-group row_sums; host adds the four columns (fp8 ones
            # descaled).  Group-A reduces run on DVE while group B finishes
            # on the PE, so only group B's reduces sit in the tail.
            nc.vector.tensor_reduce(
                t2[:, 0:1], z16a[:], axis=mybir.AxisListType.X, op=alu.add
            )
            nc.vector.tensor_reduce(
                t2[:, 1:2], z8a[:], axis=mybir.AxisListType.X, op=alu.add
            )
            nc.vector.tensor_reduce(
                t2[:, 2:3], z16b[:], axis=mybir.AxisListType.X, op=alu.add
            )
            nc.vector.tensor_reduce(
                t2[:, 3:4], z8b[:], axis=mybir.AxisListType.X, op=alu.add
            )
            nc.sync.dma_start(out=t_d[:], in_=t2[:])

    nc.compile()
    return nc


def get_nc():
    if "nc" not in _CACHE:
        _CACHE["nc"] = _build_nc()
    return _CACHE["nc"]


def build_graph_matrix(edge_index):
    """Dense normalized adjacency of the PyG-style GCNConv (self-loops +
    symmetric deg^{-1/2}); multi-edges accumulate like segment_sum does."""
    ei = np.concatenate(
        [edge_index.astype(np.int64), np.stack([np.arange(N), np.arange(N)])],
        axis=1,
    )
    src, dst = ei[0], ei[1]
    deg = np.zeros(N, np.float32)
    np.add.at(deg, dst, np.ones(len(dst), np.float32))
    dis = np.where(deg > 0, 1.0 / np.sqrt(np.maximum(deg, 1e-12)), 0.0).astype(
        np.float32
    )
    A = np.zeros((N, N), np.float32)
    np.add.at(A, (dst, src), dis[src] * dis[dst])
    return A


def build_host_inputs(inputs):
    """Per-core input maps + graph matrix + global fp8 scale."""
    f32 = np.float32
    import ml_dtypes

    bf16 = ml_dtypes.bfloat16
    e4m3 = ml_dtypes.float8_e4m3fn
    mats = np.asarray(inputs["mats"], f32)
    cyls = np.asarray(inputs["cyls"], f32)
    planes = np.asarray(inputs["planes"], f32)
    power = np.asarray(inputs["power"], f32)
    edge_index = np.asarray(inputs["edge_index"])

    A = build_graph_matrix(edge_index)

    # Block-diagonal node features with bias rows of ones: x0 = relu(S.T @ Wenc)
    ENC_K = 18
    S = np.zeros((ENC_K, N), f32)
    S[0:6, 0:6] = mats.T
    S[6, 0:6] = 1.0
    S[7:10, 6:10] = cyls.T
    S[10, 6:10] = 1.0
    S[11:15, 10:13] = planes.T
    S[15, 10:13] = 1.0
    S[16, 13] = power[0] / 10000.0
    S[17, 13] = 1.0

    Wenc = np.ascontiguousarray(
        np.concatenate(
            [
                np.asarray(inputs["W_mat"], f32),
                np.asarray(inputs["b_mat"], f32)[None, :],
                np.asarray(inputs["W_cyl"], f32),
                np.asarray(inputs["b_cyl"], f32)[None, :],
                np.asarray(inputs["W_pl"], f32),
                np.asarray(inputs["b_pl"], f32)[None, :],
                np.asarray(inputs["W_pw"], f32),
                np.asarray(inputs["b_pw"], f32)[None, :],
            ],
            axis=0,
        )
    )
    assert Wenc.shape == (ENC_K, D1)

    W_g1 = np.asarray(inputs["W_g1"], f32)
    W_g2 = np.asarray(inputs["W_g2"], f32)

    # replicated 14-node activations, transposed + chunk-swizzled for the PE
    x0 = np.maximum(S.T @ Wenc, 0.0)  # [N, D1]
    xb = np.ascontiguousarray(
        x0.T.reshape(KC, 128, N).transpose(1, 0, 2).reshape(128, KC * N)
    ).astype(bf16)

    NB = SH - NS
    per_core = []
    gmax = 0.0
    for c in range(NCORES):
        sl = slice(c * SH, (c + 1) * SH)
        w2 = W_g2[sl, 0]
        order = np.argsort(np.abs(w2))
        idx8, idx16 = order[:NS], order[NS:]
        Wv = W_g1[:, sl] * w2[None, :]  # [D1, SH]
        W8 = Wv[:, idx8]
        W16 = Wv[:, idx16]
        gmax = max(gmax, float(np.abs(W8).max()))
        per_core.append((W8, W16))
    s8 = float(2.0 ** np.floor(np.log2(224.0 / max(gmax, 1e-30))))

    def swz(w, ncols):
        return w.reshape(KC, 128, ncols).transpose(1, 0, 2).reshape(128, -1)

    in_maps = []
    for c in range(NCORES):
        W8, W16 = per_core[c]
        wv16 = np.ascontiguousarray(
            np.concatenate([xb, swz(W16.astype(bf16), NB)], axis=1)
        )
        wv8 = np.ascontiguousarray(swz((W8 * s8).astype(e4m3), NS))
        in_maps.append({"wv16": wv16, "wv8": wv8})
    return in_maps, A, s8


def combine_t(t_raw, s8):
    """Device t [14,4] = [bf16 A, fp8 A, bf16 B, fp8 B] -> one [14,1] part."""
    t = np.asarray(t_raw, np.float64)
    u = t[:, 0] + t[:, 2] + (t[:, 1] + t[:, 3]) / s8
    return u[:, None].astype(np.float32)


def epilogue(t_parts, A, inputs):
    f32 = np.float32
    b_g1 = np.asarray(inputs["b_g1"], f32)
    W_g2 = np.asarray(inputs["W_g2"], f32)
    b_g2 = np.asarray(inputs["b_g2"], f32)
    W_head = np.asarray(inputs["W_head"], f32)
    b_head = np.asarray(inputs["b_head"], f32)
    u = np.add.reduce([p.astype(f32) for p in t_parts])  # [14,1] un-aggregated
    t_full = A @ u + np.float32(b_g1 @ W_g2[:, 0])  # conv2 input = x1 @ W_g2
    x2 = A @ t_full + b_g2[0]
    y = float(x2[:, 0] @ W_head[:, 0]) + float(b_head[0])
    return np.array([y], dtype=f32)


def run_on_hw(in_maps, trace=False, tmpdir=None):
    from concourse.bass_utils import run_bass_kernel_spmd

    nc = get_nc()
    return run_bass_kernel_spmd(
        nc,
        in_maps,
        core_ids=list(range(NCORES)),
        trace=trace,
        tmpdir=tmpdir,
    )


def kernel(**inputs):
    in_maps, A, s8 = build_host_inputs(inputs)
    res = run_on_hw(in_maps, trace=bool(int(os.environ.get("KERNEL_TRACE", "0"))))
    _CACHE["last_result"] = res
    t_parts = [combine_t(r["t"], s8) for r in res.results]
    return epilogue(t_parts, A, inputs)


# revision 26
# speedup vs baseline: 1.1648x; 1.0531x over previous
"""Trainium2 Bass kernel for nn_MatSurfGcn (GCN message passing, memory-bound).

Strategy (column-parallel over W_g1's output dim, 8 cores):
  reference =  enc -> gcn_conv(W_g1) -> gcn_conv(W_g2) -> head
  Both convs are linear and A @ (X @ W) == (A @ X) @ W, so the graph
  aggregation commutes out of the device entirely.  Per core c:
    x0T = relu(enc(inputs)).T          [4096, 14]  (host, replicated)
    z_c = x0T.T @ (W_g1[:,c] * w2_c)   [14, 1024]  (device, the real work)
    t_c = row_sum(z_c)                 [14, ..]
    host: y = head(A(A Su + b1.W_g2) + b_g2)       (two 14x14 matvecs)

  W_g2 is folded into W_g1's columns on the host (same device FLOPs, kills
  the tail multiply).  The folded weight streams with column-split mixed
  precision: after the fold, column j's contribution to t scales with
  |w2[j]|, so the 512 smallest-|w2| columns per core (~13% of the sum-w2^2
  energy) stream as scaled fp8-e4m3 and the 512 largest as bf16 --
  1.5 B/elem, 6.3 MB/core, ~16.5 us of DMA at the ~380 GB/s per-core HBM
  limit, against ~14 us of PE bf16 column stream.  Measured end-to-end
  error ~5e-3 vs the 2e-2 gate (quantization noise does not average down
  through the random-sign contraction, but starts 4x under the gate).

  The 14-node activations x0T are computed on the host and replicated to
  all cores (per the sharding hint): on the PE the encoder decomposes
  into 32 latency-bound 14-column matmuls (~13 us) that would gate the
  first z matmul and starve the W stream, while as data it is a 114 KiB
  bf16 ride-along on the first W tile's DMA.
"""

import os

import numpy as np

D1, D2 = 4096, 8192
N = 14
NCORES = 8
SH = D2 // NCORES        # 1024 W_g1 columns per core
KC = D1 // 128           # 32 contraction chunks of 128
NS = 512                 # fp8 (small-|w2|) columns per core; rest bf16
# 4-chunk super-tiles keep the DMA-issue count below the stream time
# (each super-tile costs two ~0.65us HWDGE issues on one sequencer, so
# many small tiles make the head issue-bound); small tiles at the end
# shorten the post-stream matmul chain
TILE_CHUNKS = [4] * 7 + [2, 1, 1]
KSPLIT = 28              # z accumulation groups: k<28 and k>=28 (reduce overlap)
WBUFS = int(os.environ.get("KERNEL_WBUFS", "8"))
XBC = KC * N             # xb columns, prepended to the bf16 image

_CACHE = {}


def _build_nc():
    import concourse.bacc as bacc
    import concourse.bass as bass
    import concourse.mybir as mybir
    import concourse.tile as tile

    f32 = mybir.dt.float32
    bf16 = mybir.dt.bfloat16
    fp8 = mybir.dt.float8e4
    psum = bass.MemorySpace.PSUM
    alu = mybir.AluOpType

    nc = bacc.Bacc(
        "TRN2", target_bir_lowering=False, debug=False, enable_asserts=False
    )

    # host-computed x0T = relu(enc(inputs)).T (swizzled so chunk k sits at
    # cols 14k..14k+14) prepended to the host-swizzled bf16 half of the
    # folded W_g1 shard; the scaled fp8 half is a second image:
    #   wv16[p, k*N + n]       = x0T[k*128 + p, n]
    #   wv16[p, XBC + k*NB + j] = (W_g1 * w2)[k*128+p, big cols]  as bf16
    #   wv8[p, k*NS + j]        = (W_g1 * w2 * s8)[k*128+p, small cols] as fp8
    NB = SH - NS
    wv16_d = nc.dram_tensor(
        "wv16", [128, XBC + KC * NB], bf16, kind="ExternalInput"
    )
    wv8_d = nc.dram_tensor("wv8", [128, KC * NS], fp8, kind="ExternalInput")
    t_d = nc.dram_tensor("t", [N, 4], f32, kind="ExternalOutput")

    with tile.TileContext(nc) as tc:
        with (
            tc.tile_pool(name="const", bufs=1) as cpool,
            tc.tile_pool(name="wvp", bufs=WBUFS) as wpool,
            tc.tile_pool(name="zps", bufs=1, space=psum) as zps,
        ):
            t2 = cpool.tile([N, 4], f32)
            wsc = cpool.tile([128, 512], bf16)
            z16a = zps.tile([N, NB], f32)
            z8a = zps.tile([N, NS], f32)
            z16b = zps.tile([N, NB], f32)
            z8b = zps.tile([N, NS], f32)
            zw = zps.tile([N, 512], f32)

            # Tile 0 carries [xb | bf16 chunks 0..3] in ONE transfer; it
            # lives in the const pool so xb persists for all 64 stationary
            # loads.  Each super-tile is a bf16 + an fp8 transfer on the
            # same sync HWDGE ring: one ring keeps arrival order = need
            # order.
            t0w = TILE_CHUNKS[0] * NB
            tile0 = cpool.tile([128, XBC + t0w], bf16)
            nc.sync.dma_start(out=tile0[:], in_=wv16_d[:, 0 : XBC + t0w])
            xb = tile0[:, 0:XBC]
            w8_0 = wpool.tile([128, TILE_CHUNKS[0] * NS], fp8, tag="w8h")
            nc.sync.dma_start(
                out=w8_0[:], in_=wv8_d[:, 0 : TILE_CHUNKS[0] * NS]
            )
            tiles = [(tile0[:, XBC : XBC + t0w], w8_0[:], 0, TILE_CHUNKS[0])]
            k0 = TILE_CHUNKS[0]
            for nch in TILE_CHUNKS[1:]:
                w16 = wpool.tile([128, nch * NB], bf16, tag=f"w16{nch}")
                nc.sync.dma_start(
                    out=w16[:],
                    in_=wv16_d[:, XBC + k0 * NB : XBC + (k0 + nch) * NB],
                )
                w8 = wpool.tile([128, nch * NS], fp8, tag=f"w8{nch}")
                nc.sync.dma_start(
                    out=w8[:], in_=wv8_d[:, k0 * NS : (k0 + nch) * NS]
                )
                tiles.append((w16[:], w8[:], k0, nch))
                k0 += nch

            # Warm-up: the PE downclocks 2x when idle and needs ~3us of
            # continuous work to reach full speed, so burn dummy matmuls on
            # memset scratch while the first W tile is still in flight --
            # the real stream then starts at full clock.
            nc.vector.memset(wsc[:], 0.0)
            for _ in range(8):
                nc.tensor.matmul(
                    zw[:], wsc[:, 0:N], wsc[:], start=True, stop=True
                )

            # z += x0T_k.T @ Wv_k, accumulated in PSUM per (dtype, k-group);
            # the k<28 groups' reduces overlap the last chunks' matmuls
            for w16, w8, k0, nch in tiles:
                for a in range(nch):
                    k = k0 + a
                    z16 = z16a if k < KSPLIT else z16b
                    z8 = z8a if k < KSPLIT else z8b
                    st = k in (0, KSPLIT)
                    sp = k in (KSPLIT - 1, KC - 1)
                    nc.tensor.matmul(
                        z16[:],
                        xb[:, k * N : (k + 1) * N],
                        w16[:, a * NB : (a + 1) * NB],
                        start=st,
                        stop=sp,
                    )
                    nc.tensor.matmul(
                        z8[:],
                        xb[:, k * N : (k + 1) * N],
                        w8[:, a * NS : (a + 1) * NS],
                        start=st,
                        stop=sp,
                    )

            # t = per-group row_sums; host adds the four columns (fp8 ones
            # descaled).  Group-A reduces run on DVE while group B finishes
            # on the PE, so only group B's reduces sit in the tail.
            nc.vector.tensor_reduce(
                t2[:, 0:1], z16a[:], axis=mybir.AxisListType.X, op=alu.add
            )
            nc.vector.tensor_reduce(
                t2[:, 1:2], z8a[:], axis=mybir.AxisListType.X, op=alu.add
            )
            nc.vector.tensor_reduce(
                t2[:, 2:3], z16b[:], axis=mybir.AxisListType.X, op=alu.add
            )
            nc.vector.tensor_reduce(
                t2[:, 3:4], z8b[:], axis=mybir.AxisListType.X, op=alu.add
            )
            nc.sync.dma_start(out=t_d[:], in_=t2[:])

    nc.compile()
    return nc


def get_nc():
    if "nc" not in _CACHE:
        _CACHE["nc"] = _build_nc()
    return _CACHE["nc"]


def build_graph_matrix(edge_index):
    """Dense normalized adjacency of the PyG-style GCNConv (self-loops +
    symmetric deg^{-1/2}); multi-edges accumulate like segment_sum does."""
    ei = np.concatenate(
        [edge_index.astype(np.int64), np.stack([np.arange(N), np.arange(N)])],
        axis=1,
    )
    src, dst = ei[0], ei[1]
    deg = np.zeros(N, np.float32)
    np.add.at(deg, dst, np.ones(len(dst), np.float32))
    dis = np.where(deg > 0, 1.0 / np.sqrt(np.maximum(deg, 1e-12)), 0.0).astype(
        np.float32
    )
    A = np.zeros((N, N), np.float32)
    np.add.at(A, (dst, src), dis[src] * dis[dst])
    return A


def build_host_inputs(inputs):
    """Per-core input maps + graph matrix + global fp8 scale."""
    f32 = np.float32
    import ml_dtypes

    bf16 = ml_dtypes.bfloat16
    e4m3 = ml_dtypes.float8_e4m3fn
    mats = np.asarray(inputs["mats"], f32)
    cyls = np.asarray(inputs["cyls"], f32)
    planes = np.asarray(inputs["planes"], f32)
    power = np.asarray(inputs["power"], f32)
    edge_index = np.asarray(inputs["edge_index"])

    A = build_graph_matrix(edge_index)

    # Block-diagonal node features with bias rows of ones: x0 = relu(S.T @ Wenc)
    ENC_K = 18
    S = np.zeros((ENC_K, N), f32)
    S[0:6, 0:6] = mats.T
    S[6, 0:6] = 1.0
    S[7:10, 6:10] = cyls.T
    S[10, 6:10] = 1.0
    S[11:15, 10:13] = planes.T
    S[15, 10:13] = 1.0
    S[16, 13] = power[0] / 10000.0
    S[17, 13] = 1.0

    Wenc = np.ascontiguousarray(
        np.concatenate(
            [
                np.asarray(inputs["W_mat"], f32),
                np.asarray(inputs["b_mat"], f32)[None, :],
                np.asarray(inputs["W_cyl"], f32),
                np.asarray(inputs["b_cyl"], f32)[None, :],
                np.asarray(inputs["W_pl"], f32),
                np.asarray(inputs["b_pl"], f32)[None, :],
                np.asarray(inputs["W_pw"], f32),
                np.asarray(inputs["b_pw"], f32)[None, :],
            ],
            axis=0,
        )
    )
    assert Wenc.shape == (ENC_K, D1)

    W_g1 = np.asarray(inputs["W_g1"], f32)
    W_g2 = np.asarray(inputs["W_g2"], f32)

    # replicated 14-node activations, transposed + chunk-swizzled for the PE
    x0 = np.maximum(S.T @ Wenc, 0.0)  # [N, D1]
    xb = np.ascontiguousarray(
        x0.T.reshape(KC, 128, N).transpose(1, 0, 2).reshape(128, KC * N)
    ).astype(bf16)

    NB = SH - NS
    per_core = []
    gmax = 0.0
    for c in range(NCORES):
        sl = slice(c * SH, (c + 1) * SH)
        w2 = W_g2[sl, 0]
        order = np.argsort(np.abs(w2))
        idx8, idx16 = order[:NS], order[NS:]
        Wv = W_g1[:, sl] * w2[None, :]  # [D1, SH]
        W8 = Wv[:, idx8]
        W16 = Wv[:, idx16]
        gmax = max(gmax, float(np.abs(W8).max()))
        per_core.append((W8, W16))
    s8 = float(2.0 ** np.floor(np.log2(224.0 / max(gmax, 1e-30))))

    def swz(w, ncols):
        return w.reshape(KC, 128, ncols).transpose(1, 0, 2).reshape(128, -1)

    in_maps = []
    for c in range(NCORES):
        W8, W16 = per_core[c]
        wv16 = np.ascontiguousarray(
            np.concatenate([xb, swz(W16.astype(bf16), NB)], axis=1)
        )
        wv8 = np.ascontiguousarray(swz((W8 * s8).astype(e4m3), NS))
        in_maps.append({"wv16": wv16, "wv8": wv8})
    return in_maps, A, s8


def combine_t(t_raw, s8):
    """Device t [14,4] = [bf16 A, fp8 A, bf16 B, fp8 B] -> one [14,1] part."""
    t = np.asarray(t_raw, np.float64)
    u = t[:, 0] + t[:, 2] + (t[:, 1] + t[:, 3]) / s8
    return u[:, None].astype(np.float32)


def epilogue(t_parts, A, inputs):
    f32 = np.float32
    b_g1 = np.asarray(inputs["b_g1"], f32)
    W_g2 = np.asarray(inputs["W_g2"], f32)
    b_g2 = np.asarray(inputs["b_g2"], f32)
    W_head = np.asarray(inputs["W_head"], f32)
    b_head = np.asarray(inputs["b_head"], f32)
    u = np.add.reduce([p.astype(f32) for p in t_parts])  # [14,1] un-aggregated
    t_full = A @ u + np.float32(b_g1 @ W_g2[:, 0])  # conv2 input = x1 @ W_g2
    x2 = A @ t_full + b_g2[0]
    y = float(x2[:, 0] @ W_head[:, 0]) + float(b_head[0])
    return np.array([y], dtype=f32)


def run_on_hw(in_maps, trace=False, tmpdir=None):
    from concourse.bass_utils import run_bass_kernel_spmd

    nc = get_nc()
    return run_bass_kernel_spmd(
        nc,
        in_maps,
        core_ids=list(range(NCORES)),
        trace=trace,
        tmpdir=tmpdir,
    )


def kernel(**inputs):
    in_maps, A, s8 = build_host_inputs(inputs)
    res = run_on_hw(in_maps, trace=bool(int(os.environ.get("KERNEL_TRACE", "0"))))
    _CACHE["last_result"] = res
    t_parts = [combine_t(r["t"], s8) for r in res.results]
    return epilogue(t_parts, A, inputs)
